# revision 17
# baseline (speedup 1.0000x reference)
"""Trainium2 Bass kernel for nn_AutoShiftsAug.

The reference op reduces to a per-batch constant 2D translation with bilinear
resampling over a replicate-padded, zero-extended image:

    out[b,c,i,j] = sum_{ty,tx} wy[b,ty,i] * wx[b,tx]
                   * XPZ[b, c, ytap(b,ty,i), j + X0_b + tx]

with per-row-exact vertical taps and a per-batch uniform integer horizontal
tap X0_b plus fractional weight.  All tap/weight data depends only on the
tiny inputs (mean/var/eps/noise) and is computed on host; batch-sharded
across 8 cores (16 batches each).

Host prep (part of building the per-core shard layout anyway): x is
transposed to [b, i, c, w] and each batch's channel rows are stored as the
130-column padded window [X0_b, X0_b+130) of the replicate-padded,
zero-extended image — so the device sees a fixed-layout input and every
device-side access pattern is static.

Device pipeline per batch:
  1. plain DMA load G [128 rows, 9*130].
  2. TensorE: z = Wy @ G — per-batch banded vertical-blend matrix
     (host-built, exact weights incl. replicate-clamp merging and
     zero-validity) as 3 accumulation-free matmul chunks into PSUM.
  3. ScalarE/VectorE: out = wx0 * z[:, :, 0:128] + wx1 * z[:, :, 1:129].
  4. store (out in [b, i, c, w]; host transposes back).
"""

import numpy as np

PAD = 4
H = 128
HP = H + 2 * PAD  # 136
NCH = 9
NB_TOT = 128
NCORES = 8
NB = NB_TOT // NCORES  # batches per core
W2 = 130  # stored columns per channel: padded cols [X0, X0+130)
XROW = NCH * W2  # 1170
MMCHUNK = 512  # fp32 matmul moving-dim limit


# ----------------------------------------------------------------------------
# host-side parameter computation (fp32, mirroring the jax reference math)
# ----------------------------------------------------------------------------
def _host_params(mean, var, eps, noise):
    f32 = np.float32
    mean = np.asarray(mean, f32)
    var = np.asarray(var, f32)
    eps = np.asarray(eps, f32)
    noise = np.asarray(noise, f32)

    bound = f32(2.0 * (2 * PAD + 1) / HP)
    m = np.clip(mean, f32(1e-6), bound).astype(f32)
    s = np.clip(var, f32(1e-6), None).astype(f32)
    shift = np.clip(m + s * eps, f32(0.0), bound).astype(f32)  # (2,)

    ar = np.linspace(f32(-1.0 + 1.0 / HP), f32(1.0 - 1.0 / HP), HP, dtype=f32)[:H]

    def coords(a):
        g = (
            ar[None, :] + shift[a] + noise[:, 0, 0, a][:, None] + f32(1.0)
        ) * f32(HP * 0.5) - f32(0.5)
        return g.astype(f32)

    gx = coords(0)  # column axis (varies along j)
    gy = coords(1)  # row axis (varies along i)

    # vertical: per-row exact taps/weights
    a0 = np.floor(gy).astype(np.int64)
    fy = (gy - a0).astype(f32)
    v0 = ((a0 >= 0) & (a0 < HP)).astype(f32)
    v1 = ((a0 + 1 >= 0) & (a0 + 1 < HP)).astype(f32)
    wy0 = ((f32(1.0) - fy) * v0).astype(f32)
    wy1 = (fy * v1).astype(f32)
    r0 = np.clip(a0 - PAD, 0, H - 1).astype(np.int32)
    r1 = np.clip(a0 + 1 - PAD, 0, H - 1).astype(np.int32)

    # horizontal: per-batch uniform tap/weight
    d = gx - np.arange(H, dtype=f32)[None, :]
    dm = d.mean(axis=1, dtype=np.float64).astype(f32)
    X0 = np.clip(np.floor(dm).astype(np.int64), -PAD, 3 * PAD).astype(np.int32)
    fx = (dm - X0).astype(f32)

    return r0, r1, wy0, wy1, X0, fx


def _core_inputs(x, r0, r1, wy0, wy1, X0, fx, k):
    """Per-core input arrays for core k. x is the full [128,9,128,128] array."""
    b0 = k * NB
    # x shard in [b, i, c, w] layout with the per-batch padded column window
    # [X0, X0+130) of the replicate-padded, zero-extended image.
    xs = np.zeros((NB, H, NCH, W2), np.float32)
    t = np.arange(W2, dtype=np.int64)
    for bl in range(NB):
        bg = b0 + bl
        p = int(X0[bg]) + t  # padded col
        valid = (p >= 0) & (p < HP)
        cc = np.clip(p - PAD, 0, H - 1)
        img = x[bg].transpose(1, 0, 2)  # [i, c, w]
        xs[bl] = img[:, :, cc] * valid[None, None, :].astype(np.float32)

    wxp = np.zeros((H, 2 * NB), np.float32)
    wyT = np.zeros((NB, H, H), np.float32)
    r = np.arange(H, dtype=np.int64)
    for bl in range(NB):
        bg = b0 + bl
        wxp[:, 2 * bl] = 1.0 - fx[bg]
        wxp[:, 2 * bl + 1] = fx[bg]
        Wy = np.zeros((H, H), np.float32)
        np.add.at(Wy, (r, r0[bg]), wy0[bg])
        np.add.at(Wy, (r, r1[bg]), wy1[bg])
        wyT[bl] = Wy.T
    return {"x": xs.reshape(NB, H, XROW), "wxp": wxp, "wyT": wyT}


# ----------------------------------------------------------------------------
# bass program
# ----------------------------------------------------------------------------
_PROG_CACHE = {}


def _build_program():
    import concourse.bacc as bacc
    import concourse.tile as tile
    import concourse.mybir as mybir

    f32 = mybir.dt.float32
    mult = mybir.AluOpType.mult
    add = mybir.AluOpType.add

    nc = bacc.Bacc("TRN2", target_bir_lowering=False, num_devices=NCORES, debug=False)

    xd = nc.dram_tensor("x", [NB, H, XROW], f32, kind="ExternalInput")
    wxd = nc.dram_tensor("wxp", [H, 2 * NB], f32, kind="ExternalInput")
    wyd = nc.dram_tensor("wyT", [NB, H, H], f32, kind="ExternalInput")
    outd = nc.dram_tensor("out", [NB, H, NCH, H], f32, kind="ExternalOutput")

    with tile.TileContext(nc) as tc:
        with (
            tc.tile_pool(name="pp", bufs=1) as ppool,
            tc.tile_pool(name="p", bufs=4) as pool,
            tc.tile_pool(name="ps", bufs=2, space="PSUM") as psum,
        ):
            wxt_all = ppool.tile([H, 2 * NB], f32, tag="wxt")
            nc.sync.dma_start(wxt_all[:], wxd.ap())

            for b in range(NB):
                wxt = wxt_all[:, 2 * b : 2 * b + 2]
                wyt = pool.tile([H, H], f32, tag="wyt")
                nc.gpsimd.dma_start(wyt[:], wyd.ap()[b])

                # SWDGE for the big loads: splitting traffic across both DGE
                # paths raises aggregate DMA throughput vs HWDGE alone.
                g = pool.tile([H, XROW], f32, tag="g")
                nc.gpsimd.dma_start(g[:], xd.ap()[b])

                z = psum.tile([H, XROW], f32, tag="z")
                for c0 in range(0, XROW, MMCHUNK):
                    c1 = min(c0 + MMCHUNK, XROW)
                    nc.tensor.matmul(
                        out=z[:, c0:c1],
                        lhsT=wyt[:],
                        rhs=g[:, c0:c1],
                        start=True,
                        stop=True,
                    )

                zv = z[:].rearrange("p (c w) -> p c w", w=W2)
                p1 = pool.tile([H, NCH, H], f32, tag="p1")
                nc.scalar.mul(p1[:], zv[:, :, 0:H], wxt[:, 0:1])
                ot = pool.tile([H, NCH, H], f32, tag="ot")
                nc.vector.scalar_tensor_tensor(
                    out=ot[:],
                    in0=zv[:, :, 1 : H + 1],
                    scalar=wxt[:, 1:2],
                    in1=p1[:],
                    op0=mult,
                    op1=add,
                )
                st_eng = nc.sync if b % 2 == 0 else nc.gpsimd
                st_eng.dma_start(outd.ap()[b], ot[:])

    nc.compile()
    return nc


def _get_program():
    if "nc" not in _PROG_CACHE:
        _PROG_CACHE["nc"] = _build_program()
    return _PROG_CACHE["nc"]


# ----------------------------------------------------------------------------
# entry point
# ----------------------------------------------------------------------------
def kernel(x, mean, var, eps, noise):
    from concourse.bass_utils import run_bass_kernel_spmd

    x = np.ascontiguousarray(np.asarray(x, np.float32))
    params = _host_params(mean, var, eps, noise)
    in_maps = [_core_inputs(x, *params, k) for k in range(NCORES)]

    nc = _get_program()
    res = run_bass_kernel_spmd(nc, in_maps, core_ids=list(range(NCORES)))
    out = np.concatenate(
        [res.results[k]["out"].transpose(0, 2, 1, 3) for k in range(NCORES)], axis=0
    )
    return np.ascontiguousarray(out.astype(np.float32))


# revision 18
# speedup vs baseline: 1.1407x; 1.1407x over previous
"""Trainium2 Bass kernel for nn_AutoShiftsAug.

The reference op reduces to a per-batch constant 2D translation with bilinear
resampling over a replicate-padded, zero-extended image:

    out[b,c,i,j] = sum_{ty,tx} wy[b,ty,i] * wx[b,tx]
                   * XPZ[b, c, ytap(b,ty,i), j + X0_b + tx]

with per-row-exact vertical taps and a per-batch uniform integer horizontal
tap X0_b plus fractional weight.  All tap/weight data depends only on the
tiny inputs (mean/var/eps/noise) and is computed on host; batch-sharded
across 8 cores (16 batches each).

Host prep (part of building the per-core shard layout anyway): x is
transposed to [b, i, c, w] and each batch's channel rows are stored as the
130-column padded window [X0_b, X0_b+130) of the replicate-padded,
zero-extended image — so the device sees a fixed-layout input and every
device-side access pattern is static.

Device pipeline per batch:
  1. plain DMA load G [128 rows, 9*130].
  2. TensorE: z = Wy @ G — per-batch banded vertical-blend matrix
     (host-built, exact weights incl. replicate-clamp merging and
     zero-validity) as 3 accumulation-free matmul chunks into PSUM.
  3. ScalarE/VectorE: out = wx0 * z[:, :, 0:128] + wx1 * z[:, :, 1:129].
  4. store (out in [b, i, c, w]; host transposes back).
"""

import numpy as np

PAD = 4
H = 128
HP = H + 2 * PAD  # 136
NCH = 9
NB_TOT = 128
NCORES = 8
NB = NB_TOT // NCORES  # batches per core
W2 = 130  # stored columns per channel: padded cols [X0, X0+130)
XROW = NCH * W2  # 1170
MMCHUNK = 512  # fp32 matmul moving-dim limit


# ----------------------------------------------------------------------------
# host-side parameter computation (fp32, mirroring the jax reference math)
# ----------------------------------------------------------------------------
def _host_params(mean, var, eps, noise):
    f32 = np.float32
    mean = np.asarray(mean, f32)
    var = np.asarray(var, f32)
    eps = np.asarray(eps, f32)
    noise = np.asarray(noise, f32)

    bound = f32(2.0 * (2 * PAD + 1) / HP)
    m = np.clip(mean, f32(1e-6), bound).astype(f32)
    s = np.clip(var, f32(1e-6), None).astype(f32)
    shift = np.clip(m + s * eps, f32(0.0), bound).astype(f32)  # (2,)

    ar = np.linspace(f32(-1.0 + 1.0 / HP), f32(1.0 - 1.0 / HP), HP, dtype=f32)[:H]

    def coords(a):
        g = (
            ar[None, :] + shift[a] + noise[:, 0, 0, a][:, None] + f32(1.0)
        ) * f32(HP * 0.5) - f32(0.5)
        return g.astype(f32)

    gx = coords(0)  # column axis (varies along j)
    gy = coords(1)  # row axis (varies along i)

    # vertical: per-row exact taps/weights
    a0 = np.floor(gy).astype(np.int64)
    fy = (gy - a0).astype(f32)
    v0 = ((a0 >= 0) & (a0 < HP)).astype(f32)
    v1 = ((a0 + 1 >= 0) & (a0 + 1 < HP)).astype(f32)
    wy0 = ((f32(1.0) - fy) * v0).astype(f32)
    wy1 = (fy * v1).astype(f32)
    r0 = np.clip(a0 - PAD, 0, H - 1).astype(np.int32)
    r1 = np.clip(a0 + 1 - PAD, 0, H - 1).astype(np.int32)

    # horizontal: per-batch uniform tap/weight
    d = gx - np.arange(H, dtype=f32)[None, :]
    dm = d.mean(axis=1, dtype=np.float64).astype(f32)
    X0 = np.clip(np.floor(dm).astype(np.int64), -PAD, 3 * PAD).astype(np.int32)
    fx = (dm - X0).astype(f32)

    return r0, r1, wy0, wy1, X0, fx


def _core_inputs(x, r0, r1, wy0, wy1, X0, fx, k):
    """Per-core input arrays for core k. x is the full [128,9,128,128] array."""
    b0 = k * NB
    # x shard in [b, i, c, w] layout with the per-batch padded column window
    # [X0, X0+130) of the replicate-padded, zero-extended image.
    xs = np.zeros((NB, H, NCH, W2), np.float32)
    t = np.arange(W2, dtype=np.int64)
    for bl in range(NB):
        bg = b0 + bl
        p = int(X0[bg]) + t  # padded col
        valid = (p >= 0) & (p < HP)
        cc = np.clip(p - PAD, 0, H - 1)
        img = x[bg].transpose(1, 0, 2)  # [i, c, w]
        xs[bl] = img[:, :, cc] * valid[None, None, :].astype(np.float32)

    wxp = np.zeros((H, 2 * NB), np.float32)
    wyT = np.zeros((NB, H, H), np.float32)
    r = np.arange(H, dtype=np.int64)
    for bl in range(NB):
        bg = b0 + bl
        wxp[:, 2 * bl] = 1.0 - fx[bg]
        wxp[:, 2 * bl + 1] = fx[bg]
        Wy = np.zeros((H, H), np.float32)
        np.add.at(Wy, (r, r0[bg]), wy0[bg])
        np.add.at(Wy, (r, r1[bg]), wy1[bg])
        wyT[bl] = Wy.T
    return {"x": xs.reshape(NB, H, XROW), "wxp": wxp, "wyT": wyT}


# ----------------------------------------------------------------------------
# bass program
# ----------------------------------------------------------------------------
_PROG_CACHE = {}


def _build_program():
    import concourse.bacc as bacc
    import concourse.tile as tile
    import concourse.mybir as mybir

    f32 = mybir.dt.float32
    mult = mybir.AluOpType.mult
    add = mybir.AluOpType.add

    nc = bacc.Bacc("TRN2", target_bir_lowering=False, num_devices=NCORES, debug=False)

    xd = nc.dram_tensor("x", [NB, H, XROW], f32, kind="ExternalInput")
    wxd = nc.dram_tensor("wxp", [H, 2 * NB], f32, kind="ExternalInput")
    wyd = nc.dram_tensor("wyT", [NB, H, H], f32, kind="ExternalInput")
    outd = nc.dram_tensor("out", [NB, H, NCH, H], f32, kind="ExternalOutput")

    with tile.TileContext(nc) as tc:
        with (
            tc.tile_pool(name="pp", bufs=1) as ppool,
            tc.tile_pool(name="p", bufs=4) as pool,
            tc.tile_pool(name="ps", bufs=2, space="PSUM") as psum,
        ):
            wxt_all = ppool.tile([H, 2 * NB], f32, tag="wxt")
            nc.sync.dma_start(wxt_all[:], wxd.ap())

            for b in range(NB):
                wxt = wxt_all[:, 2 * b : 2 * b + 2]
                wyt = pool.tile([H, H], f32, tag="wyt")
                nc.gpsimd.dma_start(wyt[:], wyd.ap()[b])

                # SWDGE for the big loads: splitting traffic across both DGE
                # paths raises aggregate DMA throughput vs HWDGE alone.
                g = pool.tile([H, XROW], f32, tag="g")
                nc.gpsimd.dma_start(g[:], xd.ap()[b])

                z = psum.tile([H, XROW], f32, tag="z")
                for c0 in range(0, XROW, MMCHUNK):
                    c1 = min(c0 + MMCHUNK, XROW)
                    nc.tensor.matmul(
                        out=z[:, c0:c1],
                        lhsT=wyt[:],
                        rhs=g[:, c0:c1],
                        start=True,
                        stop=True,
                    )

                zv = z[:].rearrange("p (c w) -> p c w", w=W2)
                p1 = pool.tile([H, NCH, H], f32, tag="p1")
                nc.scalar.mul(p1[:], zv[:, :, 0:H], wxt[:, 0:1])
                ot = pool.tile([H, NCH, H], f32, tag="ot")
                nc.vector.scalar_tensor_tensor(
                    out=ot[:],
                    in0=zv[:, :, 1 : H + 1],
                    scalar=wxt[:, 1:2],
                    in1=p1[:],
                    op0=mult,
                    op1=add,
                )
                nc.sync.dma_start(outd.ap()[b], ot[:])

    nc.compile()
    return nc


def _get_program():
    if "nc" not in _PROG_CACHE:
        _PROG_CACHE["nc"] = _build_program()
    return _PROG_CACHE["nc"]


# ----------------------------------------------------------------------------
# entry point
# ----------------------------------------------------------------------------
def kernel(x, mean, var, eps, noise):
    from concourse.bass_utils import run_bass_kernel_spmd

    x = np.ascontiguousarray(np.asarray(x, np.float32))
    params = _host_params(mean, var, eps, noise)
    in_maps = [_core_inputs(x, *params, k) for k in range(NCORES)]

    nc = _get_program()
    res = run_bass_kernel_spmd(nc, in_maps, core_ids=list(range(NCORES)))
    out = np.concatenate(
        [res.results[k]["out"].transpose(0, 2, 1, 3) for k in range(NCORES)], axis=0
    )
    return np.ascontiguousarray(out.astype(np.float32))


# revision 19
# speedup vs baseline: 1.1856x; 1.0393x over previous
"""Trainium2 Bass kernel for nn_AutoShiftsAug.

The reference op reduces to a per-batch constant 2D translation with bilinear
resampling over a replicate-padded, zero-extended image:

    out[b,c,i,j] = sum_{ty,tx} wy[b,ty,i] * wx[b,tx]
                   * XPZ[b, c, ytap(b,ty,i), j + X0_b + tx]

with per-row-exact vertical taps and a per-batch uniform integer horizontal
tap X0_b plus fractional weight.  All tap/weight data depends only on the
tiny inputs (mean/var/eps/noise) and is computed on host; batch-sharded
across 8 cores (16 batches each).

Host prep (part of building the per-core shard layout anyway): x is
transposed to [b, i, c, w] and each batch's channel rows are stored as the
130-column padded window [X0_b, X0_b+130) of the replicate-padded,
zero-extended image — so the device sees a fixed-layout input and every
device-side access pattern is static.

Device pipeline per batch:
  1. plain DMA load G [128 rows, 9*130].
  2. TensorE: z = Wy @ G — per-batch banded vertical-blend matrix
     (host-built, exact weights incl. replicate-clamp merging and
     zero-validity) as 3 accumulation-free matmul chunks into PSUM.
  3. ScalarE/VectorE: out = wx0 * z[:, :, 0:128] + wx1 * z[:, :, 1:129].
  4. store (out in [b, i, c, w]; host transposes back).
"""

import numpy as np

PAD = 4
H = 128
HP = H + 2 * PAD  # 136
NCH = 9
NB_TOT = 128
NCORES = 8
NB = NB_TOT // NCORES  # batches per core
W2 = 130  # stored columns per channel: padded cols [X0, X0+130)
XROW = NCH * W2  # 1170
MMCHUNK = 512  # fp32 matmul moving-dim limit


# ----------------------------------------------------------------------------
# host-side parameter computation (fp32, mirroring the jax reference math)
# ----------------------------------------------------------------------------
def _host_params(mean, var, eps, noise):
    f32 = np.float32
    mean = np.asarray(mean, f32)
    var = np.asarray(var, f32)
    eps = np.asarray(eps, f32)
    noise = np.asarray(noise, f32)

    bound = f32(2.0 * (2 * PAD + 1) / HP)
    m = np.clip(mean, f32(1e-6), bound).astype(f32)
    s = np.clip(var, f32(1e-6), None).astype(f32)
    shift = np.clip(m + s * eps, f32(0.0), bound).astype(f32)  # (2,)

    ar = np.linspace(f32(-1.0 + 1.0 / HP), f32(1.0 - 1.0 / HP), HP, dtype=f32)[:H]

    def coords(a):
        g = (
            ar[None, :] + shift[a] + noise[:, 0, 0, a][:, None] + f32(1.0)
        ) * f32(HP * 0.5) - f32(0.5)
        return g.astype(f32)

    gx = coords(0)  # column axis (varies along j)
    gy = coords(1)  # row axis (varies along i)

    # vertical: per-row exact taps/weights
    a0 = np.floor(gy).astype(np.int64)
    fy = (gy - a0).astype(f32)
    v0 = ((a0 >= 0) & (a0 < HP)).astype(f32)
    v1 = ((a0 + 1 >= 0) & (a0 + 1 < HP)).astype(f32)
    wy0 = ((f32(1.0) - fy) * v0).astype(f32)
    wy1 = (fy * v1).astype(f32)
    r0 = np.clip(a0 - PAD, 0, H - 1).astype(np.int32)
    r1 = np.clip(a0 + 1 - PAD, 0, H - 1).astype(np.int32)

    # horizontal: per-batch uniform tap/weight
    d = gx - np.arange(H, dtype=f32)[None, :]
    dm = d.mean(axis=1, dtype=np.float64).astype(f32)
    X0 = np.clip(np.floor(dm).astype(np.int64), -PAD, 3 * PAD).astype(np.int32)
    fx = (dm - X0).astype(f32)

    return r0, r1, wy0, wy1, X0, fx


def _core_inputs(x, r0, r1, wy0, wy1, X0, fx, k):
    """Per-core input arrays for core k. x is the full [128,9,128,128] array."""
    b0 = k * NB
    # x shard in [b, i, c, w] layout with the per-batch padded column window
    # [X0, X0+130) of the replicate-padded, zero-extended image.
    xs = np.zeros((NB, H, NCH, W2), np.float32)
    t = np.arange(W2, dtype=np.int64)
    for bl in range(NB):
        bg = b0 + bl
        p = int(X0[bg]) + t  # padded col
        valid = (p >= 0) & (p < HP)
        cc = np.clip(p - PAD, 0, H - 1)
        img = x[bg].transpose(1, 0, 2)  # [i, c, w]
        xs[bl] = img[:, :, cc] * valid[None, None, :].astype(np.float32)

    wxp = np.zeros((H, 2 * NB), np.float32)
    wyT = np.zeros((NB, H, H), np.float32)
    r = np.arange(H, dtype=np.int64)
    for bl in range(NB):
        bg = b0 + bl
        wxp[:, 2 * bl] = 1.0 - fx[bg]
        wxp[:, 2 * bl + 1] = fx[bg]
        Wy = np.zeros((H, H), np.float32)
        np.add.at(Wy, (r, r0[bg]), wy0[bg])
        np.add.at(Wy, (r, r1[bg]), wy1[bg])
        wyT[bl] = Wy.T
    return {"x": xs.reshape(NB, H, XROW), "wxp": wxp, "wyT": wyT}


# ----------------------------------------------------------------------------
# bass program
# ----------------------------------------------------------------------------
_PROG_CACHE = {}


def _build_program():
    import concourse.bacc as bacc
    import concourse.tile as tile
    import concourse.mybir as mybir

    f32 = mybir.dt.float32
    mult = mybir.AluOpType.mult
    add = mybir.AluOpType.add

    nc = bacc.Bacc("TRN2", target_bir_lowering=False, num_devices=NCORES, debug=False)

    xd = nc.dram_tensor("x", [NB, H, XROW], f32, kind="ExternalInput")
    wxd = nc.dram_tensor("wxp", [H, 2 * NB], f32, kind="ExternalInput")
    wyd = nc.dram_tensor("wyT", [NB, H, H], f32, kind="ExternalInput")
    outd = nc.dram_tensor("out", [NB, H, NCH, H], f32, kind="ExternalOutput")

    with tile.TileContext(nc) as tc:
        with (
            tc.tile_pool(name="pp", bufs=1) as ppool,
            tc.tile_pool(name="p", bufs=4) as pool,
            tc.tile_pool(name="ps", bufs=2, space="PSUM") as psum,
        ):
            wxt_all = ppool.tile([H, 2 * NB], f32, tag="wxt")
            nc.sync.dma_start(wxt_all[:], wxd.ap())

            for b in range(NB):
                wxt = wxt_all[:, 2 * b : 2 * b + 2]
                wyt = pool.tile([H, H], f32, tag="wyt")
                nc.gpsimd.dma_start(wyt[:], wyd.ap()[b])

                # SWDGE for the big loads: splitting traffic across both DGE
                # paths raises aggregate DMA throughput vs HWDGE alone.
                g = pool.tile([H, XROW], f32, tag="g")
                nc.gpsimd.dma_start(g[:], xd.ap()[b])

                # channel-aligned chunks (3 channels each): matmul -> blend ->
                # store pipeline per chunk, so stores start before the whole
                # batch's vertical blend is done.
                CCH = 3
                for kc in range(0, NCH, CCH):
                    cw = CCH * W2
                    z = psum.tile([H, cw], f32, tag=f"z{kc}")
                    nc.tensor.matmul(
                        out=z[:],
                        lhsT=wyt[:],
                        rhs=g[:, kc * W2 : kc * W2 + cw],
                        start=True,
                        stop=True,
                    )
                    zv = z[:].rearrange("p (c w) -> p c w", w=W2)
                    p1 = pool.tile([H, CCH, H], f32, tag=f"p1{kc}")
                    nc.scalar.mul(p1[:], zv[:, :, 0:H], wxt[:, 0:1])
                    ot = pool.tile([H, CCH, H], f32, tag=f"ot{kc}")
                    nc.vector.scalar_tensor_tensor(
                        out=ot[:],
                        in0=zv[:, :, 1 : H + 1],
                        scalar=wxt[:, 1:2],
                        in1=p1[:],
                        op0=mult,
                        op1=add,
                    )
                    nc.sync.dma_start(outd.ap()[b, :, kc : kc + CCH, :], ot[:])

    nc.compile()
    return nc


def _get_program():
    if "nc" not in _PROG_CACHE:
        _PROG_CACHE["nc"] = _build_program()
    return _PROG_CACHE["nc"]


# ----------------------------------------------------------------------------
# entry point
# ----------------------------------------------------------------------------
def kernel(x, mean, var, eps, noise):
    from concourse.bass_utils import run_bass_kernel_spmd

    x = np.ascontiguousarray(np.asarray(x, np.float32))
    params = _host_params(mean, var, eps, noise)
    in_maps = [_core_inputs(x, *params, k) for k in range(NCORES)]

    nc = _get_program()
    res = run_bass_kernel_spmd(nc, in_maps, core_ids=list(range(NCORES)))
    out = np.concatenate(
        [res.results[k]["out"].transpose(0, 2, 1, 3) for k in range(NCORES)], axis=0
    )
    return np.ascontiguousarray(out.astype(np.float32))
